# revision 15
# baseline (speedup 1.0000x reference)
"""Trainium2 Bass kernel for nn_DiffusionGraphConv (gnn_message_passing).

Reference computation (B=64, N=1024, D=128=64+64, O=128, 2 supports,
2 diffusion steps):
    x0 = concat(inputs, state)                      # [B, N, D]
    y1 = S0 x0 ; z2 = S0 y1 ; y3 = S1 y1 ; z4 = S1 y3
    xs = [x0, y1, 2 z2 - x0, y3, 2 z4 - y1]
    out = concat_d(xs) @ W + bias                   # [B*N, O]

Algebraic refactor (host folds the +-/2x into the weight blocks, and the
feature projection commutes with the node-space supports):
    Wa = W0 - W2, Wb = W1 - W4, Wc = 2 W2, Wd = W3, We = 2 W4
    out = x0 Wa + S0 (x0 Wb + y1 Wc) + S1 (y1 Wd + y3 We)

Sharding: data-parallel over batch, 8 batches per NeuronCore, supports and
weights replicated. Per-core device schedule (all accumulation in fp32 PSUM):
    pass1: y1   = S0 x0                              (float32r, 128 MMs @512)
    trans: y1T  = PE-transpose(y1) per (b, ntile)    (f32r -> bf16)
    pass2: y3T  = (S1 y1)^T  via reversed operands   (float32r, 128 MMs @512)
    feat:  P1 = x0 Wb + y1 Wc ; P2 = y1 Wd + y3 We   (bf16 MMs, K=128)
    final: out = x0 Wa + S0 P1 + S1 P2 + bias        (f32r + bf16 MMs)
P1 reuses x0's SBUF slots, P2 reuses y1's (their lifetimes are disjoint).
"""
import sys

if "/opt/trn_rl_repo" not in sys.path:
    sys.path.insert(0, "/opt/trn_rl_repo")

import numpy as np
import ml_dtypes

import concourse.bass as bass
import concourse.mybir as mybir
from concourse import bacc, tile
from concourse.bass_utils import run_bass_kernel_spmd
from concourse.masks import make_identity

N_CORES = 8
B = 64
BL = B // N_CORES          # local batches per core
N = 1024                   # nodes
D = 128                    # input_size (64 input + 64 hidden)
O = 128                    # output_size
NT = N // 128              # node partition tiles
F32R = mybir.dt.float32r
BF16 = mybir.dt.bfloat16
F32 = mybir.dt.float32

_CACHE = {}


def _build(debug_taps=False, reps=1):
    nc = bacc.Bacc("TRN2", target_bir_lowering=False, debug=False,
                   num_devices=N_CORES)
    s0t_d = nc.dram_tensor("s0t", [N, N], F32R, kind="ExternalInput").ap()
    s1t_d = nc.dram_tensor("s1t", [N, N], F32R, kind="ExternalInput").ap()
    x0f_d = nc.dram_tensor("x0f", [N, BL * D], F32R, kind="ExternalInput").ap()
    x0t_d = nc.dram_tensor("x0t", [BL * D, N], BF16, kind="ExternalInput").ap()
    wf_d = nc.dram_tensor("wf", [5 * D, O], BF16, kind="ExternalInput").ap()
    bias_d = nc.dram_tensor("biasb", [128, 512], F32, kind="ExternalInput").ap()
    out_d = nc.dram_tensor("out", [N, BL, O], F32, kind="ExternalOutput").ap()
    if debug_taps:
        y1f_dbg = nc.dram_tensor("y1f_dbg", [N, BL * D], F32R,
                                 kind="ExternalOutput").ap()
        y1t_dbg = nc.dram_tensor("y1t_dbg", [BL * D, N], BF16,
                                 kind="ExternalOutput").ap()
        y3t_dbg = nc.dram_tensor("y3t_dbg", [BL * D, N], BF16,
                                 kind="ExternalOutput").ap()
        p1f_dbg = nc.dram_tensor("p1f_dbg", [N, BL * O], F32R,
                                 kind="ExternalOutput").ap()
        p2f_dbg = nc.dram_tensor("p2f_dbg", [N, BL * O], F32R,
                                 kind="ExternalOutput").ap()

    with tile.TileContext(nc) as tc:
        with (
            tc.tile_pool(name="main", bufs=1) as mp,
            tc.tile_pool(name="outp", bufs=4) as op,
            tc.tile_pool(name="psb", bufs=4, space="PSUM") as pb,
            tc.tile_pool(name="pss", bufs=4, space="PSUM") as psm,
        ):
            # ---- persistent SBUF residents (loaded once) ----
            s0t = []
            s1t = []
            for j in range(NT):
                t = mp.tile([128, N], F32R, tag=f"s0t{j}", name=f"s0t{j}")
                nc.sync.dma_start(out=t[:], in_=s0t_d[j * 128:(j + 1) * 128, :])
                s0t.append(t)
            x0t = []
            for b in range(BL):
                t = mp.tile([128, N], BF16, tag=f"x0t{b}", name=f"x0t{b}")
                nc.sync.dma_start(out=t[:], in_=x0t_d[b * 128:(b + 1) * 128, :])
                x0t.append(t)
            for j in range(NT):
                t = mp.tile([128, N], F32R, tag=f"s1t{j}", name=f"s1t{j}")
                nc.sync.dma_start(out=t[:], in_=s1t_d[j * 128:(j + 1) * 128, :])
                s1t.append(t)
            w = []
            for k in range(5):
                t = mp.tile([128, O], BF16, tag=f"w{k}", name=f"w{k}")
                nc.sync.dma_start(out=t[:], in_=wf_d[k * 128:(k + 1) * 128, :])
                w.append(t)
            bias_t = mp.tile([128, 512], F32, tag="bias")
            nc.sync.dma_start(out=bias_t[:], in_=bias_d[:])
            identf = mp.tile([128, 128], F32, tag="idf")
            make_identity(nc, identf[:])
            identr = mp.tile([128, 128], F32R, tag="idr")
            nc.vector.tensor_copy(identr[:], identf[:])

            ci = 0

            def pcopy(dst, src):
                # alternate DVE / ACT for PSUM->SBUF moves
                nonlocal ci
                if ci % 2 == 0:
                    nc.vector.tensor_copy(dst, src)
                else:
                    nc.scalar.copy(dst, src)
                ci += 1

            for rep in range(reps):
                # ---- x0 F-layout load (slots reused by P1 at rep end) ----
                xa = []
                for j in range(NT):
                    t = mp.tile([128, BL * D], F32R, tag=f"xa{j}",
                                name=f"xa{j}_{rep}")
                    nc.sync.dma_start(
                        out=t[:], in_=x0f_d[j * 128:(j + 1) * 128, :])
                    xa.append(t)
                yb = [mp.tile([128, BL * D], F32R, tag=f"yb{j}",
                              name=f"yb{j}_{rep}") for j in range(NT)]
                y1t = [mp.tile([128, N], BF16, tag=f"y1t{b}",
                               name=f"y1t{b}_{rep}") for b in range(BL)]
                y3t = [mp.tile([128, N], BF16, tag=f"y3t{b}",
                               name=f"y3t{b}_{rep}") for b in range(BL)]

                # ---- pass 1: y1 = S0 x0, F-layout [n, (b,d)] + transposes ----
                # jt-outer with two live PSUM tiles: each stationary S-tile
                # load serves both 512-wide halves.
                for it in range(NT):
                    ps0 = pb.tile([128, 512], F32, tag="big", name=f"ps0_{rep}_{it}")
                    ps1 = pb.tile([128, 512], F32, tag="big", name=f"ps1_{rep}_{it}")
                    for jt in range(NT):
                        lhs = s0t[jt][:, it * 128:(it + 1) * 128]
                        nc.tensor.matmul(ps0[:], lhs, xa[jt][:, 0:512],
                                         start=(jt == 0), stop=(jt == NT - 1))
                        nc.tensor.matmul(ps1[:], lhs, xa[jt][:, 512:1024],
                                         start=(jt == 0), stop=(jt == NT - 1))
                    for f, ps in ((0, ps0), (1, ps1)):
                        pcopy(yb[it][:, f * 512:(f + 1) * 512], ps[:])
                        # transposes for the 4 batches in this window
                        for b in range(4 * f, 4 * f + 4):
                            pst = psm.tile([128, 128], F32R, tag="tr")
                            nc.tensor.transpose(
                                pst[:], yb[it][:, b * 128:(b + 1) * 128],
                                identr[:])
                            pcopy(y1t[b][:, it * 128:(it + 1) * 128], pst[:])

                # ---- pass 2: y3T = (S1 y1)^T, T-layout [(b,d), n] ----
                # jt-outer: each y1-block stationary load serves both halves.
                for b in range(BL):
                    ps0 = pb.tile([128, 512], F32, tag="big", name=f"p2a_{rep}_{b}")
                    ps1 = pb.tile([128, 512], F32, tag="big", name=f"p2b_{rep}_{b}")
                    for jt in range(NT):
                        lhs = yb[jt][:, b * 128:(b + 1) * 128]
                        nc.tensor.matmul(ps0[:], lhs, s1t[jt][:, 0:512],
                                         start=(jt == 0), stop=(jt == NT - 1))
                        nc.tensor.matmul(ps1[:], lhs, s1t[jt][:, 512:1024],
                                         start=(jt == 0), stop=(jt == NT - 1))
                    for f2, ps in ((0, ps0), (1, ps1)):
                        pcopy(y3t[b][:, f2 * 512:(f2 + 1) * 512], ps[:])

                if debug_taps:
                    for j in range(NT):
                        sl = slice(j * 128, (j + 1) * 128)
                        nc.sync.dma_start(out=y1f_dbg[sl, :], in_=yb[j][:])

                # ---- feature projections ----
                # P1 = x0 Wb + y1 Wc   (into x0f's slots)
                # P2 = y1 Wd + y3 We   (into y1's slots)
                p1f = []
                p2f = []
                for nt in range(NT):
                    t1 = mp.tile([128, BL * O], F32R, tag=f"xa{nt}",
                                 name=f"p1f{nt}_{rep}")
                    t2 = mp.tile([128, BL * O], F32R, tag=f"yb{nt}",
                                 name=f"p2f{nt}_{rep}")
                    for h in range(2):
                        ps = pb.tile([128, 512], F32, tag="big")
                        for q, b in enumerate(range(4 * h, 4 * h + 4)):
                            reg = ps[:, q * 128:(q + 1) * 128]
                            nc.tensor.matmul(
                                reg, x0t[b][:, nt * 128:(nt + 1) * 128],
                                w[1][:], start=True, stop=False)
                            nc.tensor.matmul(
                                reg, y1t[b][:, nt * 128:(nt + 1) * 128],
                                w[2][:], start=False, stop=True)
                        pcopy(t1[:, h * 512:(h + 1) * 512], ps[:])
                        ps = pb.tile([128, 512], F32, tag="big")
                        for q, b in enumerate(range(4 * h, 4 * h + 4)):
                            reg = ps[:, q * 128:(q + 1) * 128]
                            nc.tensor.matmul(
                                reg, y1t[b][:, nt * 128:(nt + 1) * 128],
                                w[3][:], start=True, stop=False)
                            nc.tensor.matmul(
                                reg, y3t[b][:, nt * 128:(nt + 1) * 128],
                                w[4][:], start=False, stop=True)
                        pcopy(t2[:, h * 512:(h + 1) * 512], ps[:])
                    p1f.append(t1)
                    p2f.append(t2)

                if debug_taps:
                    for j in range(NT):
                        sl = slice(j * 128, (j + 1) * 128)
                        nc.sync.dma_start(out=p1f_dbg[sl, :], in_=p1f[j][:])
                        nc.sync.dma_start(out=p2f_dbg[sl, :], in_=p2f[j][:])
                    for b in range(BL):
                        sl = slice(b * 128, (b + 1) * 128)
                        nc.sync.dma_start(out=y1t_dbg[sl, :], in_=y1t[b][:])
                        nc.sync.dma_start(out=y3t_dbg[sl, :], in_=y3t[b][:])

                # ---- final: out = x0 Wa + S0 P1 + S1 P2 + bias ----
                # jt-outer, two live PSUM tiles per it. 16 full-tile matmuls
                # first: start=True on the first clears the bank has_written
                # bits once; everything after accumulates per-element.
                for it in range(NT):
                    ps0 = pb.tile([128, 512], F32, tag="big",
                                  name=f"fin0_{rep}_{it}")
                    ps1 = pb.tile([128, 512], F32, tag="big",
                                  name=f"fin1_{rep}_{it}")
                    for jt in range(NT):
                        lhs = s0t[jt][:, it * 128:(it + 1) * 128]
                        nc.tensor.matmul(ps0[:], lhs, p1f[jt][:, 0:512],
                                         start=(jt == 0), stop=False,
                                         skip_group_check=True)
                        nc.tensor.matmul(ps1[:], lhs, p1f[jt][:, 512:1024],
                                         start=(jt == 0), stop=False,
                                         skip_group_check=True)
                    for jt in range(NT):
                        lhs = s1t[jt][:, it * 128:(it + 1) * 128]
                        nc.tensor.matmul(ps0[:], lhs, p2f[jt][:, 0:512],
                                         start=False, stop=False,
                                         skip_group_check=True)
                        nc.tensor.matmul(ps1[:], lhs, p2f[jt][:, 512:1024],
                                         start=False, stop=False,
                                         skip_group_check=True)
                    # x0*Wa region-adds LAST, start=False: no bank clear.
                    for f, ps in ((0, ps0), (1, ps1)):
                        for q, b in enumerate(range(4 * f, 4 * f + 4)):
                            nc.tensor.matmul(
                                ps[:, q * 128:(q + 1) * 128],
                                x0t[b][:, it * 128:(it + 1) * 128], w[0][:],
                                start=False, stop=(q == 3),
                                skip_group_check=True)
                        ot = op.tile([128, 512], F32, tag="out")
                        nc.vector.tensor_add(ot[:], ps[:], bias_t[:])
                        nc.sync.dma_start(
                            out=out_d[it * 128:(it + 1) * 128,
                                      4 * f:4 * f + 4, :],
                            in_=ot[:])
    nc.compile()
    return nc


def _prep_inputs(supports, inputs, state, weight, biases):
    supports = np.asarray(supports, dtype=np.float32)
    inputs = np.asarray(inputs, dtype=np.float32)
    state = np.asarray(state, dtype=np.float32)
    weight = np.asarray(weight, dtype=np.float32)
    biases = np.asarray(biases, dtype=np.float32)

    s0t = np.ascontiguousarray(supports[0].T)
    s1t = np.ascontiguousarray(supports[1].T)

    x0 = np.concatenate(
        [inputs.reshape(B, N, D // 2), state.reshape(B, N, D // 2)], axis=2)
    x0f = np.ascontiguousarray(x0.transpose(1, 0, 2))      # [N, B, D]
    x0t = np.ascontiguousarray(x0.transpose(0, 2, 1))      # [B, D, N]
    x0t_bf = x0t.astype(ml_dtypes.bfloat16)

    W = weight.reshape(5, D, O)
    wf = np.concatenate([
        W[0] - W[2],        # Wa
        W[1] - W[4],        # Wb
        2.0 * W[2],         # Wc
        W[3],               # Wd
        2.0 * W[4],         # We
    ], axis=0).astype(ml_dtypes.bfloat16)

    biasb = np.ascontiguousarray(np.tile(biases[None, :], (128, 4)))

    in_maps = []
    for c in range(N_CORES):
        bsl = slice(c * BL, (c + 1) * BL)
        in_maps.append({
            "s0t": s0t,
            "s1t": s1t,
            "x0f": np.ascontiguousarray(x0f[:, bsl, :]).reshape(N, BL * D),
            "x0t": np.ascontiguousarray(x0t_bf[bsl]).reshape(BL * D, N),
            "wf": wf,
            "biasb": biasb,
        })
    return in_maps


def _get_runner(reps=1):
    """Build the jitted SPMD executor once (mirrors
    bass2jax.run_bass_via_pjrt) so repeated calls don't re-trace."""
    if ("runner", reps) in _CACHE:
        return _CACHE[("runner", reps)]
    import jax
    from jax.sharding import Mesh, PartitionSpec, NamedSharding
    from concourse import bass2jax
    import concourse.mybir as mb

    if ("nc", reps) not in _CACHE:
        _CACHE[("nc", reps)] = _build(reps=reps)
    nc = _CACHE[("nc", reps)]
    bass2jax.install_neuronx_cc_hook()

    part_name = nc.partition_id_tensor.name if nc.partition_id_tensor else None
    in_names, out_names, out_avals, zero_outs = [], [], [], []
    for alloc in nc.m.functions[0].allocations:
        if not isinstance(alloc, mb.MemoryLocationSet):
            continue
        name = alloc.memorylocations[0].name
        if alloc.kind == "ExternalInput":
            if name != part_name:
                in_names.append(name)
        elif alloc.kind == "ExternalOutput":
            out_names.append(name)
            shape = tuple(alloc.tensor_shape)
            dtype = mb.dt.np(alloc.dtype)
            out_avals.append(jax.core.ShapedArray(shape, dtype))
            zero_outs.append(np.zeros(shape, dtype))
    n_params = len(in_names)
    all_names = in_names + out_names
    if part_name is not None:
        all_names = all_names + [part_name]

    def _body(*args):
        operands = list(args)
        if part_name is not None:
            operands.append(bass2jax.partition_id_tensor())
        outs = bass2jax._bass_exec_p.bind(
            *operands,
            out_avals=tuple(out_avals),
            in_names=tuple(all_names),
            out_names=tuple(out_names),
            lowering_input_output_aliases=(),
            sim_require_finite=True,
            sim_require_nnan=True,
            nc=nc,
        )
        return tuple(outs)

    devices = jax.devices()[:N_CORES]
    mesh = Mesh(np.asarray(devices), ("core",))
    from jax.experimental.shard_map import shard_map
    n_outs = len(out_names)
    donate = tuple(range(n_params, n_params + n_outs))
    sharded = jax.jit(
        shard_map(_body, mesh=mesh,
                  in_specs=(PartitionSpec("core"),) * (n_params + n_outs),
                  out_specs=(PartitionSpec("core"),) * n_outs,
                  check_rep=False),
        donate_argnums=donate, keep_unused=True)
    sh = NamedSharding(mesh, PartitionSpec("core"))

    runner = {
        "fn": sharded, "in_names": in_names, "out_names": out_names,
        "zero_outs": zero_outs, "sharding": sh, "mesh": mesh,
    }
    _CACHE[("runner", reps)] = runner
    return runner


def _run(in_maps, device_inputs=None, reps=1):
    """Execute on the 8 cores; returns list of per-core output dicts."""
    import jax
    r = _get_runner(reps)
    if device_inputs is None:
        device_inputs = _put_inputs(in_maps, reps)
    zeros = [
        jax.device_put(
            np.zeros((N_CORES * z.shape[0], *z.shape[1:]), z.dtype),
            r["sharding"])
        for z in r["zero_outs"]
    ]
    out_arrs = r["fn"](*device_inputs, *zeros)
    outs = [np.asarray(a) for a in out_arrs]
    return [
        {name: outs[i].reshape(N_CORES, *r["zero_outs"][i].shape)[c]
         for i, name in enumerate(r["out_names"])}
        for c in range(N_CORES)
    ]


def _put_inputs(in_maps, reps=1):
    import jax
    r = _get_runner(reps)
    return [
        jax.device_put(
            np.concatenate([np.asarray(in_maps[c][n]) for c in range(N_CORES)],
                           axis=0), r["sharding"])
        for n in r["in_names"]
    ]


def kernel(supports, inputs, state, weight, biases, output_size=O, **_):
    assert int(output_size) == O
    in_maps = _prep_inputs(supports, inputs, state, weight, biases)
    res = _run(in_maps)
    # per-core out: [N, BL, O] -> full [B, N*O]
    outs = np.stack([res[c]["out"] for c in range(N_CORES)])
    out = outs.transpose(0, 2, 1, 3).reshape(B, N * O)
    return np.ascontiguousarray(out)


if __name__ == "__main__":
    rng = np.random.default_rng(0)
    sup = rng.standard_normal((2, N, N)).astype(np.float32) / np.sqrt(N)
    inp = rng.standard_normal((B, N * 64)).astype(np.float32)
    st = rng.standard_normal((B, N * 64)).astype(np.float32)
    wt = rng.standard_normal((5 * D, O)).astype(np.float32) * 0.05
    bs = np.zeros((O,), np.float32)
    out = kernel(sup, inp, st, wt, bs, O)
    print("out", out.shape, out.dtype, float(np.abs(out).max()))


# revision 17
# speedup vs baseline: 1.1811x; 1.1811x over previous
"""Trainium2 Bass kernel for nn_DiffusionGraphConv (gnn_message_passing).

Reference computation (B=64, N=1024, D=128=64+64, O=128, 2 supports,
2 diffusion steps):
    x0 = concat(inputs, state)                      # [B, N, D]
    y1 = S0 x0 ; z2 = S0 y1 ; y3 = S1 y1 ; z4 = S1 y3
    xs = [x0, y1, 2 z2 - x0, y3, 2 z4 - y1]
    out = concat_d(xs) @ W + bias                   # [B*N, O]

Algebraic refactor (host folds the +-/2x into the weight blocks, and the
feature projection commutes with the node-space supports):
    Wa = W0 - W2, Wb = W1 - W4, Wc = 2 W2, Wd = W3, We = 2 W4
    out = x0 Wa + S0 (x0 Wb + y1 Wc) + S1 (y1 Wd + y3 We)

Sharding: data-parallel over batch, 8 batches per NeuronCore, supports and
weights replicated. Per-core device schedule (all accumulation in fp32 PSUM):
    pass1: y1   = S0 x0                              (float32r, 128 MMs @512)
    trans: y1T  = PE-transpose(y1) per (b, ntile)    (f32r -> bf16)
    pass2: y3T  = (S1 y1)^T  via reversed operands   (float32r, 128 MMs @512)
    feat:  P1 = x0 Wb + y1 Wc ; P2 = y1 Wd + y3 We   (bf16 MMs, K=128)
    final: out = x0 Wa + S0 P1 + S1 P2 + bias        (f32r + bf16 MMs)
P1 reuses x0's SBUF slots, P2 reuses y1's (their lifetimes are disjoint).
"""
import sys

if "/opt/trn_rl_repo" not in sys.path:
    sys.path.insert(0, "/opt/trn_rl_repo")

import numpy as np
import ml_dtypes

import concourse.bass as bass
import concourse.mybir as mybir
from concourse import bacc, tile
from concourse.bass_utils import run_bass_kernel_spmd
from concourse.masks import make_identity

N_CORES = 8
B = 64
BL = B // N_CORES          # local batches per core
N = 1024                   # nodes
D = 128                    # input_size (64 input + 64 hidden)
O = 128                    # output_size
NT = N // 128              # node partition tiles
F32R = mybir.dt.float32r
BF16 = mybir.dt.bfloat16
F32 = mybir.dt.float32

_CACHE = {}


def _build(debug_taps=False, reps=1):
    nc = bacc.Bacc("TRN2", target_bir_lowering=False, debug=False,
                   num_devices=N_CORES)
    s0t_d = nc.dram_tensor("s0t", [N, N], F32R, kind="ExternalInput").ap()
    s1t_d = nc.dram_tensor("s1t", [N, N], F32R, kind="ExternalInput").ap()
    x0f_d = nc.dram_tensor("x0f", [N, BL * D], F32R, kind="ExternalInput").ap()
    x0t_d = nc.dram_tensor("x0t", [BL * D, N], BF16, kind="ExternalInput").ap()
    wf_d = nc.dram_tensor("wf", [5 * D, O], BF16, kind="ExternalInput").ap()
    bias_d = nc.dram_tensor("biasb", [128, 512], F32, kind="ExternalInput").ap()
    out_d = nc.dram_tensor("out", [N, BL, O], F32, kind="ExternalOutput").ap()
    if debug_taps:
        y1f_dbg = nc.dram_tensor("y1f_dbg", [N, BL * D], F32R,
                                 kind="ExternalOutput").ap()
        y1t_dbg = nc.dram_tensor("y1t_dbg", [BL * D, N], BF16,
                                 kind="ExternalOutput").ap()
        y3t_dbg = nc.dram_tensor("y3t_dbg", [BL * D, N], BF16,
                                 kind="ExternalOutput").ap()
        p1f_dbg = nc.dram_tensor("p1f_dbg", [N, BL * O], F32R,
                                 kind="ExternalOutput").ap()
        p2f_dbg = nc.dram_tensor("p2f_dbg", [N, BL * O], F32R,
                                 kind="ExternalOutput").ap()

    with tile.TileContext(nc) as tc:
        with (
            tc.tile_pool(name="main", bufs=1) as mp,
            tc.tile_pool(name="outp", bufs=4) as op,
            tc.tile_pool(name="psb", bufs=4, space="PSUM") as pb,
            tc.tile_pool(name="pss", bufs=4, space="PSUM") as psm,
        ):
            # ---- persistent SBUF residents (loaded once) ----
            s0t = []
            s1t = []
            for j in range(NT):
                t = mp.tile([128, N], F32R, tag=f"s0t{j}", name=f"s0t{j}")
                nc.sync.dma_start(out=t[:], in_=s0t_d[j * 128:(j + 1) * 128, :])
                s0t.append(t)
            x0t = []
            for b in range(BL):
                t = mp.tile([128, N], BF16, tag=f"x0t{b}", name=f"x0t{b}")
                nc.sync.dma_start(out=t[:], in_=x0t_d[b * 128:(b + 1) * 128, :])
                x0t.append(t)
            for j in range(NT):
                t = mp.tile([128, N], F32R, tag=f"s1t{j}", name=f"s1t{j}")
                nc.sync.dma_start(out=t[:], in_=s1t_d[j * 128:(j + 1) * 128, :])
                s1t.append(t)
            w = []
            for k in range(5):
                t = mp.tile([128, O], BF16, tag=f"w{k}", name=f"w{k}")
                nc.sync.dma_start(out=t[:], in_=wf_d[k * 128:(k + 1) * 128, :])
                w.append(t)
            bias_t = mp.tile([128, 512], F32, tag="bias")
            nc.sync.dma_start(out=bias_t[:], in_=bias_d[:])
            identf = mp.tile([128, 128], F32, tag="idf")
            make_identity(nc, identf[:])
            identr = mp.tile([128, 128], F32R, tag="idr")
            nc.vector.tensor_copy(identr[:], identf[:])

            ci = 0

            def pcopy(dst, src):
                # alternate DVE / ACT for PSUM->SBUF moves
                nonlocal ci
                if ci % 2 == 0:
                    nc.vector.tensor_copy(dst, src)
                else:
                    nc.scalar.copy(dst, src)
                ci += 1

            for rep in range(reps):
                # ---- x0 F-layout load (slots reused by P1 at rep end) ----
                xa = []
                for j in range(NT):
                    t = mp.tile([128, BL * D], F32R, tag=f"xa{j}",
                                name=f"xa{j}_{rep}")
                    nc.sync.dma_start(
                        out=t[:], in_=x0f_d[j * 128:(j + 1) * 128, :])
                    xa.append(t)
                yb = [mp.tile([128, BL * D], F32R, tag=f"yb{j}",
                              name=f"yb{j}_{rep}") for j in range(NT)]
                y1t = [mp.tile([128, N], BF16, tag=f"y1t{b}",
                               name=f"y1t{b}_{rep}") for b in range(BL)]
                y3t = [mp.tile([128, N], BF16, tag=f"y3t{b}",
                               name=f"y3t{b}_{rep}") for b in range(BL)]

                # ---- pass 1: y1 = S0 x0, F-layout [n, (b,d)] + transposes ----
                for it in range(NT):
                    for f in range(2):
                        ps = pb.tile([128, 512], F32, tag="big")
                        for jt in range(NT):
                            nc.tensor.matmul(
                                ps[:],
                                s0t[jt][:, it * 128:(it + 1) * 128],
                                xa[jt][:, f * 512:(f + 1) * 512],
                                start=(jt == 0), stop=(jt == NT - 1),
                            )
                        pcopy(yb[it][:, f * 512:(f + 1) * 512], ps[:])
                        # transposes for the 4 batches in this window
                        for b in range(4 * f, 4 * f + 4):
                            pst = psm.tile([128, 128], F32R, tag="tr")
                            nc.tensor.transpose(
                                pst[:], yb[it][:, b * 128:(b + 1) * 128],
                                identr[:])
                            pcopy(y1t[b][:, it * 128:(it + 1) * 128], pst[:])

                # ---- pass 2: y3T = (S1 y1)^T, T-layout [(b,d), n] ----
                for b in range(BL):
                    for f2 in range(2):
                        ps = pb.tile([128, 512], F32, tag="big")
                        for jt in range(NT):
                            nc.tensor.matmul(
                                ps[:],
                                yb[jt][:, b * 128:(b + 1) * 128],
                                s1t[jt][:, f2 * 512:(f2 + 1) * 512],
                                start=(jt == 0), stop=(jt == NT - 1),
                            )
                        pcopy(y3t[b][:, f2 * 512:(f2 + 1) * 512], ps[:])

                if debug_taps:
                    for j in range(NT):
                        sl = slice(j * 128, (j + 1) * 128)
                        nc.sync.dma_start(out=y1f_dbg[sl, :], in_=yb[j][:])

                # ---- feature projections ----
                # P1 = x0 Wb + y1 Wc   (into x0f's slots)
                # P2 = y1 Wd + y3 We   (into y1's slots)
                p1f = []
                p2f = []
                for nt in range(NT):
                    t1 = mp.tile([128, BL * O], F32R, tag=f"xa{nt}",
                                 name=f"p1f{nt}_{rep}")
                    t2 = mp.tile([128, BL * O], F32R, tag=f"yb{nt}",
                                 name=f"p2f{nt}_{rep}")
                    for h in range(2):
                        ps = pb.tile([128, 512], F32, tag="big")
                        for q, b in enumerate(range(4 * h, 4 * h + 4)):
                            reg = ps[:, q * 128:(q + 1) * 128]
                            nc.tensor.matmul(
                                reg, x0t[b][:, nt * 128:(nt + 1) * 128],
                                w[1][:], start=True, stop=False)
                            nc.tensor.matmul(
                                reg, y1t[b][:, nt * 128:(nt + 1) * 128],
                                w[2][:], start=False, stop=True)
                        pcopy(t1[:, h * 512:(h + 1) * 512], ps[:])
                        ps = pb.tile([128, 512], F32, tag="big")
                        for q, b in enumerate(range(4 * h, 4 * h + 4)):
                            reg = ps[:, q * 128:(q + 1) * 128]
                            nc.tensor.matmul(
                                reg, y1t[b][:, nt * 128:(nt + 1) * 128],
                                w[3][:], start=True, stop=False)
                            nc.tensor.matmul(
                                reg, y3t[b][:, nt * 128:(nt + 1) * 128],
                                w[4][:], start=False, stop=True)
                        pcopy(t2[:, h * 512:(h + 1) * 512], ps[:])
                    p1f.append(t1)
                    p2f.append(t2)

                if debug_taps:
                    for j in range(NT):
                        sl = slice(j * 128, (j + 1) * 128)
                        nc.sync.dma_start(out=p1f_dbg[sl, :], in_=p1f[j][:])
                        nc.sync.dma_start(out=p2f_dbg[sl, :], in_=p2f[j][:])
                    for b in range(BL):
                        sl = slice(b * 128, (b + 1) * 128)
                        nc.sync.dma_start(out=y1t_dbg[sl, :], in_=y1t[b][:])
                        nc.sync.dma_start(out=y3t_dbg[sl, :], in_=y3t[b][:])

                # ---- final: out = x0 Wa + S0 P1 + S1 P2 + bias ----
                for it in range(NT):
                    for f in range(2):
                        ps = pb.tile([128, 512], F32, tag="big",
                                     name=f"fin_{rep}_{it}_{f}")
                        # 16 full-tile matmuls first: start=True on the first
                        # clears the bank has_written bits once; everything
                        # after accumulates per-element.
                        for jt in range(NT):
                            nc.tensor.matmul(
                                ps[:],
                                s0t[jt][:, it * 128:(it + 1) * 128],
                                p1f[jt][:, f * 512:(f + 1) * 512],
                                start=(jt == 0), stop=False,
                                skip_group_check=True)
                        for jt in range(NT):
                            nc.tensor.matmul(
                                ps[:],
                                s1t[jt][:, it * 128:(it + 1) * 128],
                                p2f[jt][:, f * 512:(f + 1) * 512],
                                start=False, stop=False,
                                skip_group_check=True)
                        # x0*Wa region-adds LAST, start=False: no bank clear.
                        for q, b in enumerate(range(4 * f, 4 * f + 4)):
                            nc.tensor.matmul(
                                ps[:, q * 128:(q + 1) * 128],
                                x0t[b][:, it * 128:(it + 1) * 128], w[0][:],
                                start=False, stop=(q == 3),
                                skip_group_check=True)
                        ot = op.tile([128, 512], F32, tag="out")
                        nc.vector.tensor_add(ot[:], ps[:], bias_t[:])
                        nc.sync.dma_start(
                            out=out_d[it * 128:(it + 1) * 128,
                                      4 * f:4 * f + 4, :],
                            in_=ot[:])
    nc.compile()
    return nc


def _prep_inputs(supports, inputs, state, weight, biases):
    supports = np.asarray(supports, dtype=np.float32)
    inputs = np.asarray(inputs, dtype=np.float32)
    state = np.asarray(state, dtype=np.float32)
    weight = np.asarray(weight, dtype=np.float32)
    biases = np.asarray(biases, dtype=np.float32)

    s0t = np.ascontiguousarray(supports[0].T)
    s1t = np.ascontiguousarray(supports[1].T)

    x0 = np.concatenate(
        [inputs.reshape(B, N, D // 2), state.reshape(B, N, D // 2)], axis=2)
    x0f = np.ascontiguousarray(x0.transpose(1, 0, 2))      # [N, B, D]
    x0t = np.ascontiguousarray(x0.transpose(0, 2, 1))      # [B, D, N]
    x0t_bf = x0t.astype(ml_dtypes.bfloat16)

    W = weight.reshape(5, D, O)
    wf = np.concatenate([
        W[0] - W[2],        # Wa
        W[1] - W[4],        # Wb
        2.0 * W[2],         # Wc
        W[3],               # Wd
        2.0 * W[4],         # We
    ], axis=0).astype(ml_dtypes.bfloat16)

    biasb = np.ascontiguousarray(np.tile(biases[None, :], (128, 4)))

    in_maps = []
    for c in range(N_CORES):
        bsl = slice(c * BL, (c + 1) * BL)
        in_maps.append({
            "s0t": s0t,
            "s1t": s1t,
            "x0f": np.ascontiguousarray(x0f[:, bsl, :]).reshape(N, BL * D),
            "x0t": np.ascontiguousarray(x0t_bf[bsl]).reshape(BL * D, N),
            "wf": wf,
            "biasb": biasb,
        })
    return in_maps


def _get_runner(reps=1):
    """Build the jitted SPMD executor once (mirrors
    bass2jax.run_bass_via_pjrt) so repeated calls don't re-trace."""
    if ("runner", reps) in _CACHE:
        return _CACHE[("runner", reps)]
    import jax
    from jax.sharding import Mesh, PartitionSpec, NamedSharding
    from concourse import bass2jax
    import concourse.mybir as mb

    try:
        jax.config.update("jax_compilation_cache_dir", "/tmp/jax_cache")
        jax.config.update("jax_persistent_cache_min_compile_time_secs", 1.0)
    except Exception:
        pass

    if ("nc", reps) not in _CACHE:
        _CACHE[("nc", reps)] = _build(reps=reps)
    nc = _CACHE[("nc", reps)]
    bass2jax.install_neuronx_cc_hook()

    part_name = nc.partition_id_tensor.name if nc.partition_id_tensor else None
    in_names, out_names, out_avals, zero_outs = [], [], [], []
    for alloc in nc.m.functions[0].allocations:
        if not isinstance(alloc, mb.MemoryLocationSet):
            continue
        name = alloc.memorylocations[0].name
        if alloc.kind == "ExternalInput":
            if name != part_name:
                in_names.append(name)
        elif alloc.kind == "ExternalOutput":
            out_names.append(name)
            shape = tuple(alloc.tensor_shape)
            dtype = mb.dt.np(alloc.dtype)
            out_avals.append(jax.core.ShapedArray(shape, dtype))
            zero_outs.append(np.zeros(shape, dtype))
    n_params = len(in_names)
    all_names = in_names + out_names
    if part_name is not None:
        all_names = all_names + [part_name]

    def _body(*args):
        operands = list(args)
        if part_name is not None:
            operands.append(bass2jax.partition_id_tensor())
        outs = bass2jax._bass_exec_p.bind(
            *operands,
            out_avals=tuple(out_avals),
            in_names=tuple(all_names),
            out_names=tuple(out_names),
            lowering_input_output_aliases=(),
            sim_require_finite=True,
            sim_require_nnan=True,
            nc=nc,
        )
        return tuple(outs)

    devices = jax.devices()[:N_CORES]
    mesh = Mesh(np.asarray(devices), ("core",))
    from jax.experimental.shard_map import shard_map
    n_outs = len(out_names)
    donate = tuple(range(n_params, n_params + n_outs))
    sharded = jax.jit(
        shard_map(_body, mesh=mesh,
                  in_specs=(PartitionSpec("core"),) * (n_params + n_outs),
                  out_specs=(PartitionSpec("core"),) * n_outs,
                  check_rep=False),
        donate_argnums=donate, keep_unused=True)
    sh = NamedSharding(mesh, PartitionSpec("core"))

    runner = {
        "fn": sharded, "in_names": in_names, "out_names": out_names,
        "zero_outs": zero_outs, "sharding": sh, "mesh": mesh,
    }
    _CACHE[("runner", reps)] = runner
    return runner


def _run(in_maps, device_inputs=None, reps=1):
    """Execute on the 8 cores; returns list of per-core output dicts."""
    import jax
    r = _get_runner(reps)
    if device_inputs is None:
        device_inputs = _put_inputs(in_maps, reps)
    zeros = [
        jax.device_put(
            np.zeros((N_CORES * z.shape[0], *z.shape[1:]), z.dtype),
            r["sharding"])
        for z in r["zero_outs"]
    ]
    out_arrs = r["fn"](*device_inputs, *zeros)
    outs = [np.asarray(a) for a in out_arrs]
    return [
        {name: outs[i].reshape(N_CORES, *r["zero_outs"][i].shape)[c]
         for i, name in enumerate(r["out_names"])}
        for c in range(N_CORES)
    ]


def _put_inputs(in_maps, reps=1):
    import jax
    r = _get_runner(reps)
    return [
        jax.device_put(
            np.concatenate([np.asarray(in_maps[c][n]) for c in range(N_CORES)],
                           axis=0), r["sharding"])
        for n in r["in_names"]
    ]


def kernel(supports, inputs, state, weight, biases, output_size=O, **_):
    assert int(output_size) == O
    in_maps = _prep_inputs(supports, inputs, state, weight, biases)
    res = _run(in_maps)
    # per-core out: [N, BL, O] -> full [B, N*O]
    outs = np.stack([res[c]["out"] for c in range(N_CORES)])
    out = outs.transpose(0, 2, 1, 3).reshape(B, N * O)
    return np.ascontiguousarray(out)


if __name__ == "__main__":
    rng = np.random.default_rng(0)
    sup = rng.standard_normal((2, N, N)).astype(np.float32) / np.sqrt(N)
    inp = rng.standard_normal((B, N * 64)).astype(np.float32)
    st = rng.standard_normal((B, N * 64)).astype(np.float32)
    wt = rng.standard_normal((5 * D, O)).astype(np.float32) * 0.05
    bs = np.zeros((O,), np.float32)
    out = kernel(sup, inp, st, wt, bs, O)
    print("out", out.shape, out.dtype, float(np.abs(out).max()))
